# revision 8
# baseline (speedup 1.0000x reference)
"""ChunkDropout forward on 8 Trainium2 NeuronCores.

out = x * (1 - mask) * scaling_factor, where mask is the deterministic
chunk-dropout mask drawn from jax.random.key(42) (length-L boolean).

Strategy: pure data parallel. The mask depends only on a hardcoded RNG key,
so it is computed once on host CPU; (1 - mask) * scaling_factor folds into a
single [1, L] f32 vector replicated to every core. x is sharded 4096 -> 8 x
512 rows. Per core the Bass kernel broadcasts the scale vector across the
128 SBUF partitions once, then streams [128, 2500] tiles: DMA load ->
in-place tensor_mul on the vector engine -> DMA store. Memory-bound; roughly
41 MB of HBM traffic per core.
"""

import numpy as np

BATCH = 4096
L = 10000
N_CORES = 8
ROWS = BATCH // N_CORES  # 512 rows per core
P = 128                  # SBUF partitions
RB = ROWS // P           # 4 row blocks per core
FCH = 2500               # free-dim chunk
NF = L // FCH            # 4 chunks

# Mask hyperparameters (must match the reference module exactly)
DROPOUT_P = 0.01
HOLE_LOC = 10.0
HOLE_SCALE = 3.0
MIN_HOLE = 1

_cache = {}


def _chunk_mask_np():
    """The chunk-dropout mask for jax.random.key(42), computed on host CPU.

    Faithful translation of the reference sampler: geometric gap offsets and
    truncated/clamped normal hole lengths, sequentially OR-ed into a length-L
    boolean mask. Runs eagerly on CPU jax so device placement never touches
    the TRN backend.
    """
    if "mask" in _cache:
        return _cache["mask"]
    import jax
    import jax.numpy as jnp

    with jax.default_device(jax.devices("cpu")[0]):
        key = jax.random.key(42)
        log1mp = float(np.log(np.float32(1.0 - DROPOUT_P)))
        mask = np.zeros(L, dtype=bool)
        last_end = 0
        while True:
            key, kg, kn = jax.random.split(key, 3)
            u = float(
                jax.random.uniform(
                    kg, (), dtype=jnp.float32,
                    minval=float(np.finfo(np.float32).tiny), maxval=1.0,
                )
            )
            offset = int(np.floor(np.float32(np.log(np.float32(u))) / np.float32(log1mp)))
            offset = max(offset, 1)
            gap_start = last_end + offset
            if gap_start >= L - 1:
                break
            glen = int(np.int32(
                float(jax.random.normal(kn, (), dtype=jnp.float32)) * HOLE_SCALE + HOLE_LOC
            ))
            glen = max(glen, MIN_HOLE)
            gap_end = min(gap_start + glen, L)
            mask[gap_start:gap_end] = True
            last_end = gap_end
            if gap_end >= L:
                break
    _cache["mask"] = mask
    return mask


K = 8                    # SBUF tile slots in flight
T = RB * NF              # 16 tiles per core


def _build_nc():
    """Hand-scheduled raw-Bass pipeline.

    The walrus codegen here encodes at most ONE sync wait per instruction, so
    the schedule is built so every instruction needs at most one:
      - SP (sync/HWDGE)  issues loads, gated on store progress (slot reuse)
      - DVE              multiplies in place, gated on load progress only —
                         the WAR hazard vs the slot's previous store is covered
                         transitively because the load was issue-gated on it
      - ACT (scalar/HWDGE) issues stores, gated on multiply progress
    All loads count one semaphore, all stores another, multiplies a third, so
    every gate is a single cumulative wait.
    """
    if "nc" in _cache:
        return _cache["nc"]
    import concourse.bass as bass
    import concourse.mybir as mybir

    nc = bass.Bass()
    x = nc.declare_dram_parameter("x", [ROWS, L], mybir.dt.float32, isOutput=False)
    s = nc.declare_dram_parameter("s", [1, L], mybir.dt.float32, isOutput=False)
    out = nc.declare_dram_parameter("out", [ROWS, L], mybir.dt.float32, isOutput=True)

    f32 = mybir.dt.float32
    with (
        nc.sbuf_tensor([P, L], f32) as scale_sb,
        nc.sbuf_tensor([P, 1], f32) as probe,
        nc.sbuf_tensor([P, K, FCH], f32) as tiles,
        nc.semaphore("bcast_sem") as bcast_sem,
        nc.semaphore("load_sem") as load_sem,
        nc.semaphore("store_sem") as store_sem,
        nc.semaphore("vec_sem") as vec_sem,
        nc.Block() as block,
    ):
        def chunk(t):
            i, j = divmod(t, NF)
            rows = slice(i * P, (i + 1) * P)
            cols = slice(j * FCH, (j + 1) * FCH)
            return rows, cols, j

        @block.gpsimd
        def _(g):
            # Replicate the [1, L] scale vector into all 128 partitions by
            # log-doubling SBUF->SBUF copies: one 40 KB HBM read total; the
            # ~5 MB of replication traffic stays on the SBUF fabric instead
            # of competing with the x loads for HBM bandwidth.
            g.dma_start(out=scale_sb[0:1, :], in_=s[:, :]).then_inc(bcast_sem, 16)
            m, step = 1, 1
            while m < P:
                n = min(m, P - m)
                g.wait_ge(bcast_sem, 16 * step)
                g.dma_start(
                    out=scale_sb[m:m + n, :], in_=scale_sb[0:n, :]
                ).then_inc(bcast_sem, 16)
                m += n
                step += 1

        @block.sync
        def _(sp):
            for t in range(T):
                if t >= K:
                    sp.wait_ge(store_sem, 16 * (t - K + 1))
                rows, cols, _ = chunk(t)
                sp.dma_start(
                    out=tiles[:, t % K, :], in_=x[rows, cols]
                ).then_inc(load_sem, 16)

        @block.vector
        def _(v):
            # Absorb the broadcast wait into a throwaway copy so no later
            # instruction ever carries two fused waits.
            v.wait_ge(bcast_sem, 16 * 8)
            v.tensor_copy(out=probe[:, :], in_=scale_sb[:, 0:1])
            for t in range(T):
                v.wait_ge(load_sem, 16 * (t + 1))
                _, cols, _ = chunk(t)
                v.tensor_mul(
                    out=tiles[:, t % K, :],
                    in0=tiles[:, t % K, :],
                    in1=scale_sb[:, cols],
                )
                # DVE pipe-drain fence: this op can't issue until the mul's
                # 8-slice pipe has emptied, so its inc proves the mul's SBUF
                # writes are visible to the store DMA (the mul's own then_inc
                # can fire before the write pipe drains).
                v.tensor_copy(out=probe[:, :], in_=probe[:, :]).then_inc(vec_sem, 1)

        @block.scalar
        def _(a):
            for t in range(T):
                a.wait_ge(vec_sem, t + 1)
                rows, cols, _ = chunk(t)
                a.dma_start(
                    out=out[rows, cols], in_=tiles[:, t % K, :]
                ).then_inc(store_sem, 16)

    _cache["nc"] = nc
    return nc


def kernel(x: np.ndarray, scaling_factor: np.ndarray, **run_kwargs) -> np.ndarray:
    from concourse.bass_utils import run_bass_kernel_spmd

    mask = _chunk_mask_np()
    scale_vec = ((1.0 - mask.astype(np.float32))
                 * np.float32(scaling_factor.reshape(-1)[0])).astype(np.float32)
    scale_vec = np.ascontiguousarray(scale_vec.reshape(1, L))

    x = np.ascontiguousarray(np.asarray(x, dtype=np.float32))
    nc = _build_nc()
    in_maps = [
        {"x": x[c * ROWS:(c + 1) * ROWS], "s": scale_vec} for c in range(N_CORES)
    ]
    res = run_bass_kernel_spmd(nc, in_maps, core_ids=list(range(N_CORES)), **run_kwargs)
    out = np.concatenate([r["out"] for r in res.results], axis=0)
    if "exec_time_ns" in dir(res):
        _cache["last_result"] = res
    return out


# revision 38
# speedup vs baseline: 1.4168x; 1.4168x over previous
"""ChunkDropout forward on 8 Trainium2 NeuronCores.

out = x * (1 - mask) * scaling_factor, where mask is the deterministic
chunk-dropout mask drawn from jax.random.key(42) (length-L boolean).

Strategy: pure data parallel. The mask depends only on a hardcoded RNG key,
so it is computed once on host CPU; (1 - mask) * scaling_factor folds into a
single [1, L] f32 vector replicated to every core. x is sharded 4096 -> 8 x
512 rows. Per core the Bass kernel broadcasts the scale row across the 128
partitions with K=1 TensorEngine matmuls into PSUM (bit-exact, zero DMA
traffic), then streams [128, 2000] tiles: DMA load -> in-place tensor_mul
(in1 from PSUM) -> DMA store. Memory-bound: 41 MB HBM traffic per core,
~110-115 us vs the 114 us roofline at 358 GB/s.
"""

import numpy as np

BATCH = 4096
L = 10000
N_CORES = 8
ROWS = BATCH // N_CORES  # 512 rows per core
P = 128                  # SBUF partitions
RB = ROWS // P           # 4 row blocks per core
FCH = 2000               # free-dim chunk
NF = L // FCH            # 5 chunks
MMN = 500                # matmul free-dim per PSUM bank (<=512)
NMM = FCH // MMN         # 4 matmuls per scale chunk

# Mask hyperparameters (must match the reference module exactly)
DROPOUT_P = 0.01
HOLE_LOC = 10.0
HOLE_SCALE = 3.0
MIN_HOLE = 1

_cache = {}


def _chunk_mask_np():
    """The chunk-dropout mask for jax.random.key(42), computed on host CPU.

    Faithful translation of the reference sampler: geometric gap offsets and
    truncated/clamped normal hole lengths, sequentially OR-ed into a length-L
    boolean mask. Runs eagerly on CPU jax so device placement never touches
    the TRN backend.
    """
    if "mask" in _cache:
        return _cache["mask"]
    import jax
    import jax.numpy as jnp

    with jax.default_device(jax.devices("cpu")[0]):
        key = jax.random.key(42)
        log1mp = float(np.log(np.float32(1.0 - DROPOUT_P)))
        mask = np.zeros(L, dtype=bool)
        last_end = 0
        while True:
            key, kg, kn = jax.random.split(key, 3)
            u = float(
                jax.random.uniform(
                    kg, (), dtype=jnp.float32,
                    minval=float(np.finfo(np.float32).tiny), maxval=1.0,
                )
            )
            offset = int(np.floor(np.float32(np.log(np.float32(u))) / np.float32(log1mp)))
            offset = max(offset, 1)
            gap_start = last_end + offset
            if gap_start >= L - 1:
                break
            glen = int(np.int32(
                float(jax.random.normal(kn, (), dtype=jnp.float32)) * HOLE_SCALE + HOLE_LOC
            ))
            glen = max(glen, MIN_HOLE)
            gap_end = min(gap_start + glen, L)
            mask[gap_start:gap_end] = True
            last_end = gap_end
            if gap_end >= L:
                break
    _cache["mask"] = mask
    return mask


K = 8                    # SBUF tile slots in flight
T = RB * NF              # 20 tiles per core
LOAD_DEPTH = 4           # max in-flight x loads


def _build_nc():
    """Hand-scheduled raw-Bass pipeline.

    The walrus codegen here encodes at most ONE sync wait per instruction, so
    the schedule is built so every instruction needs at most one:
      - SP (sync/HWDGE)  issues x loads, gated per-slot on store progress
      - PE               broadcasts the scale row into PSUM via K=1 matmuls
                         against a ones vector (verified bit-exact for f32),
                         costing ZERO DMA-fabric traffic
      - DVE              multiplies in place (in1 read from PSUM), gated on
                         load progress only — the WAR hazard vs the slot's
                         previous store is covered transitively because the
                         load was issue-gated on it
      - ACT (scalar/HWDGE) issues stores, gated on multiply progress
    Per-slot DMA semaphores keep every completion count exact (a cumulative
    count over concurrent DMAs is unsound: the 16 SDMA engines skew, so later
    DMAs' fast engines can satisfy a threshold while one engine still lags —
    observed on HW as 8-partition stripes of stale data).
    """
    if "nc" in _cache:
        return _cache["nc"]
    import concourse.bass as bass
    import concourse.mybir as mybir

    nc = bass.Bass()
    x = nc.declare_dram_parameter("x", [ROWS, L], mybir.dt.float32, isOutput=False)
    s = nc.declare_dram_parameter("s", [1, L], mybir.dt.float32, isOutput=False)
    out = nc.declare_dram_parameter("out", [ROWS, L], mybir.dt.float32, isOutput=True)

    f32 = mybir.dt.float32
    from contextlib import ExitStack
    with ExitStack() as ctx:
        srow = ctx.enter_context(nc.sbuf_tensor([1, L], f32))
        ones = ctx.enter_context(nc.sbuf_tensor([1, P], f32))
        probe = ctx.enter_context(nc.sbuf_tensor([P, 64], f32))
        fence_n = [0]

        tiles = ctx.enter_context(nc.sbuf_tensor([P, K, FCH], f32))
        # Two PSUM chunk buffers, 4 banks each (matmul outputs are placed at
        # 512-f32 bank-aligned offsets; only the first MMN=500 of each bank
        # hold data). Chunk j lives in psum[j % 2].
        psum = [
            ctx.enter_context(nc.psum_tensor(f"ps{i}", [P, NMM * 512], f32))
            for i in range(2)
        ]
        srow_sem = ctx.enter_context(nc.semaphore("srow_sem"))
        ones_sem = ctx.enter_context(nc.semaphore("ones_sem"))
        pe_sem = ctx.enter_context(nc.semaphore("pe_sem"))
        vec_sem = ctx.enter_context(nc.semaphore("vec_sem"))
        slot_sems = [
            ctx.enter_context(nc.semaphore(f"slot{j}_sem")) for j in range(K)
        ]
        block = ctx.enter_context(nc.Block())

        def chunk(t):
            # Column-major traversal: tiles 0..RB-1 all use scale chunk 0, so
            # the pipeline starts as soon as the first PSUM chunk is ready.
            j, i = divmod(t, RB)
            rows = slice(i * P, (i + 1) * P)
            cols = slice(j * FCH, (j + 1) * FCH)
            return rows, cols, j

        def psum_view(j):
            # [P, NMM, MMN] view of chunk j's live columns.
            return psum[j % 2][:, :].rearrange(
                "p (n c) -> p n c", c=512
            )[:, :, 0:MMN]

        @block.sync
        def _(sp):
            sp.dma_start(out=srow[:, :], in_=s[:, :]).then_inc(srow_sem, 16)
            for t in range(T):
                if t >= LOAD_DEPTH:
                    # Cap in-flight loads: the SDMA engines round-robin all
                    # queued DMAs at packet granularity, so an unbounded
                    # backlog makes the FIRST tile finish as late as the
                    # last. One in-flight load already saturates HBM.
                    d = t - LOAD_DEPTH
                    sp.wait_ge(slot_sems[d % K], 32 * (d // K) + 16)
                if t >= K:
                    # Slot is free once its previous store completed.
                    sp.wait_ge(slot_sems[t % K], 32 * (t // K))
                rows, cols, _ = chunk(t)
                sp.dma_start(
                    out=tiles[:, t % K, :], in_=x[rows, cols]
                ).then_inc(slot_sems[t % K], 16)

        @block.tensor
        def _(te):
            te.wait_ge(srow_sem, 16)
            te.wait_ge(ones_sem, 2)
            for j in range(NF):
                if j >= 2:
                    # Chunk j reuses chunk j-2's banks: wait until group j-2
                    # muls (their PSUM reads) are done. Never concurrent with
                    # a DVE read of the same bank (HW fatal).
                    te.wait_ge(vec_sem, RB * (j - 1))
                for n in range(NMM):
                    c0 = j * FCH + n * MMN
                    nc.tensor.matmul(
                        out=psum[j % 2][:, n * 512:n * 512 + MMN],
                        lhsT=ones[:, :],
                        rhs=srow[:, c0:c0 + MMN],
                    ).then_inc(pe_sem, 1)
            # Dummy matmul: one-op margin for the last group's PSUM-write
            # visibility. Writes a single element of psum[1], whose last
            # readers (group NF-2) are gated complete first.
            te.wait_ge(vec_sem, RB * (NF - 1))
            nc.tensor.matmul(
                out=psum[1][:, 0:1], lhsT=ones[:, :], rhs=srow[:, 0:1]
            ).then_inc(pe_sem, 1)

        @block.vector
        def _(v):
            # ones for the PE broadcast; two fence copies so the PE's wait
            # implies the memset's writes are visible.
            def fence():
                # Write-only op on a fresh column: no data dependency, but
                # it can only issue after the previous DVE op's pipe drains.
                k = fence_n[0]; fence_n[0] += 1
                return v.memset(probe[:, k:k + 1], 0.0)
            v.memset(ones[:, :], 1.0)
            fence().then_inc(ones_sem, 1)
            fence().then_inc(ones_sem, 1)
            for t in range(T):
                g = t // RB
                if t % RB == 0:
                    # First consumer of scale chunk g: absorb the PE gate in
                    # a throwaway copy so no instruction carries two fused
                    # waits. +1 matmul of margin covers PSUM-write
                    # visibility (the chunk's own 4 matmuls plus the next
                    # chunk's first, or the dummy for the last chunk).
                    v.wait_ge(pe_sem, NMM * (g + 1) + 1)
                    fence()
                v.wait_ge(slot_sems[t % K], 32 * (t // K) + 16)
                v.tensor_mul(
                    out=tiles[:, t % K, :].rearrange(
                        "p (n c) -> p n c", c=MMN
                    ),
                    in0=tiles[:, t % K, :].rearrange(
                        "p (n c) -> p n c", c=MMN
                    ),
                    in1=psum_view(g),
                ).then_inc(vec_sem, 1)
            # One extra inc so the last store's +1 safety margin can be met;
            # it can only issue once the last mul's write pipe has drained.
            fence().then_inc(vec_sem, 1)

        @block.scalar
        def _(a):
            for t in range(T):
                # t+2, not t+1: one full DVE op of safety margin between the
                # mul's completion signal and the store's SBUF read.
                a.wait_ge(vec_sem, t + 2)
                rows, cols, _ = chunk(t)
                a.dma_start(
                    out=out[rows, cols], in_=tiles[:, t % K, :]
                ).then_inc(slot_sems[t % K], 16)
            # Tail: don't let the kernel finish before every store has landed
            # in DRAM.
            for j in range(K):
                cycles = (T - 1 - j) // K + 1
                a.wait_ge(slot_sems[j], 32 * cycles)

    _cache["nc"] = nc
    return nc


def kernel(x: np.ndarray, scaling_factor: np.ndarray, **run_kwargs) -> np.ndarray:
    from concourse.bass_utils import run_bass_kernel_spmd

    mask = _chunk_mask_np()
    scale_vec = ((1.0 - mask.astype(np.float32))
                 * np.float32(scaling_factor.reshape(-1)[0])).astype(np.float32)
    scale_vec = np.ascontiguousarray(scale_vec.reshape(1, L))

    x = np.ascontiguousarray(np.asarray(x, dtype=np.float32))
    nc = _build_nc()
    in_maps = [
        {"x": x[c * ROWS:(c + 1) * ROWS], "s": scale_vec} for c in range(N_CORES)
    ]
    res = run_bass_kernel_spmd(nc, in_maps, core_ids=list(range(N_CORES)), **run_kwargs)
    out = np.concatenate([r["out"] for r in res.results], axis=0)
    if "exec_time_ns" in dir(res):
        _cache["last_result"] = res
    return out
